# revision 1
# baseline (speedup 1.0000x reference)
"""Trainium2 Bass kernel for nn_GAT_77704548319854 — SBUF-dst scatter variant.

Same math as kernel.py (attention collapses to a membership-masked dense
layer). Difference: the membership counts are scatter-added directly into
SBUF via the parity-CCE mode of dma_scatter_add (sbuf_tokens_per_rank=128),
so there is no DRAM maskbuf, no zero-fill DMA gating the first chunk, and
no mask-extraction DMA in the tail. Node r of a core maps to out partition
r%128 / tile column r//128, which is exactly the mode's idx decoding:
partition = idx&127, rank slot = idx>>7 = tile t, parity = t&1 routes even
tiles to the A accumulators and odd tiles to B, group column = t//2.
Chunks rotate over 4 (A,B) accumulator pairs to stay WAW-independent;
mask = min(sum of pairs, 1), applied as strided per-parity multiplies.
"""
import sys

sys.path.insert(0, "/opt/trn_rl_repo")

import numpy as np

import concourse.bacc as bacc
import concourse.bass as bass
import concourse.mybir as mybir
import concourse.tile as tile
from concourse.bass_utils import run_bass_kernel_spmd
from concourse.masks import make_identity

F = 64
N_CORES = 8
NPC = 12544     # nodes per core (128 * 98)
GRP = NPC // 256                             # 49 group columns per parity


class Cfg:
    def __init__(self, k=25, chunk=7936, tail=2816):
        # 8192-idx chunks crash the Q7 scatter ucode (16 KB idx staging);
        # 7936 verified on HW
        self.chunk = chunk
        self.chunks = [chunk] * k + ([tail] if tail else [])
        self.cap = sum(self.chunks)          # padded idxs per core
        self.tiles = NPC // 128              # 98
        self.np_total = N_CORES * NPC


# per-core edges: mean 200704, seed-0 max 201088; cap 25*7936+2816 = 201216
FULL = Cfg()

f32 = mybir.dt.float32
bf16 = mybir.dt.bfloat16
i16 = mybir.dt.int16
IODT = bf16
WITH_CC = True


def build(cfg: Cfg, n_cores=N_CORES):
    tiles = cfg.tiles
    ids_cols = cfg.cap // 16

    nc = bacc.Bacc("TRN2", target_bir_lowering=False, debug=False,
                   num_devices=n_cores)
    ids_d = nc.dram_tensor("ids", [16, ids_cols], i16, kind="ExternalInput")
    emb_d = nc.dram_tensor("emb", [128, tiles * F], IODT,
                           kind="ExternalInput")
    wa_d = nc.dram_tensor("wa", [F, F], IODT, kind="ExternalInput")
    ba_d = nc.dram_tensor("ba", [1, F], IODT, kind="ExternalInput")
    out_d = nc.dram_tensor("out", [128, tiles * F], IODT,
                           kind="ExternalOutput")

    with tile.TileContext(nc) as tc:
        with tc.tile_pool(name="sb", bufs=1) as sb, \
             tc.tile_pool(name="sbt", bufs=2) as sbt, \
             tc.tile_pool(name="ps", bufs=2, space="PSUM") as ps, \
             tc.tile_pool(name="dram", bufs=1, space="DRAM") as dram:

            # ======== phase A: membership counts scattered into SBUF ======
            # 4 rotating (A=even-parity, B=odd-parity) accumulator pairs;
            # column GRP (49) is the dump column pad idxs land in
            nrot = 4
            acc = []
            for r in range(nrot):
                a = sb.tile([128, GRP + 1], f32, name=f"mA{r}")
                b = sb.tile([128, GRP + 1], f32, name=f"mB{r}")
                nc.vector.memset(a[:], 0.0)
                nc.vector.memset(b[:], 0.0)
                acc.append((a, b))

            idx16 = sb.tile([128, ids_cols], i16)
            c0 = cfg.chunk // 16             # chunk 0's ids staged first
            for cs in (slice(0, c0), slice(c0, ids_cols)):
                for g in range(2):
                    nc.sync.dma_start(out=idx16[16 * g:16 * (g + 1), cs],
                                      in_=ids_d[:, cs])

            ones = sb.tile([128, cfg.chunk // 128], f32)
            nc.vector.memset(ones[:], 1.0)

            if WITH_CC:
                # NEFFs containing a collective ride the runtime's fast
                # completion path (~35 ms less dispatch wall per execution)
                cc_in = dram.tile([128], f32, name="ccin")
                cc_out = dram.tile([128], f32, name="ccout")
                cz = sb.tile([128, 1], f32)
                nc.vector.memset(cz[:], 0.0)
                nc.sync.dma_start(out=cc_in[:].rearrange("(p x) -> p x",
                                                         p=128),
                                  in_=cz[:])
                nc.gpsimd.collective_compute(
                    "AllReduce", mybir.AluOpType.add,
                    replica_groups=[list(range(n_cores))],
                    ins=[cc_in[:]], outs=[cc_out[:]])

            coff = 0
            for i, sz in enumerate(cfg.chunks):
                a, b = acc[i % nrot]
                nc.gpsimd.dma_scatter_add(
                    a[:], ones[:, :sz // 128][:, :, None],
                    idx16[:, coff:coff + sz // 16],
                    sz, sz, 1,
                    sbuf_tokens_per_rank=128, parity_reg=0,
                    out_ap_other=b[:])
                coff += sz // 16

            # ======== phase B: dense per-node compute (overlaps phase A) ===
            emb_sb = sb.tile([128, tiles * F], IODT)
            nc.sync.dma_start(out=emb_sb[:], in_=emb_d[:])
            emb3 = emb_sb[:].rearrange("p (t f) -> p t f", f=F)
            out_sb = sb.tile([128, tiles * F], IODT)
            sq = sb.tile([128, tiles * F], f32)
            nc.vector.tensor_mul(out=sq[:], in0=emb_sb[:], in1=emb_sb[:])
            ssq = sb.tile([128, tiles], f32)
            nc.vector.tensor_reduce(out=ssq[:],
                                    in_=sq[:].rearrange("p (t f) -> p t f",
                                                        f=F),
                                    axis=mybir.AxisListType.X,
                                    op=mybir.AluOpType.add)
            nrm = sb.tile([128, tiles], f32)
            nc.scalar.sqrt(out=nrm[:], in_=ssq[:])
            nc.vector.tensor_scalar_add(out=nrm[:], in0=nrm[:], scalar1=1e-7)
            rec = sb.tile([128, tiles], f32)
            nc.vector.reciprocal(out=rec[:], in_=nrm[:])
            recb = sb.tile([128, tiles], IODT)
            nc.vector.tensor_scalar_min(out=recb[:], in0=rec[:], scalar1=1.0)
            nc.vector.tensor_tensor(
                out=emb3, in0=emb3,
                in1=recb[:][:, :, None].to_broadcast([128, tiles, F]),
                op=mybir.AluOpType.mult)

            ident = sb.tile([128, 128], IODT)
            make_identity(nc, ident[:])
            wat_sb = sb.tile([F, F], IODT)
            nc.sync.dma_start(out=wat_sb[:], in_=wa_d[:])
            ba_sb = sb.tile([1, F], IODT)
            nc.sync.dma_start(out=ba_sb[:], in_=ba_d[:])
            ones1 = sb.tile([1, 128], IODT)
            nc.vector.memset(ones1[:], 1.0)

            htall = sb.tile([F, tiles * 128], IODT)
            for t in range(tiles):
                h_t = emb_sb[:, t * F:(t + 1) * F]
                ht_ps = ps.tile([F, 128], IODT, tag="ht", bufs=4)
                nc.tensor.transpose(out=ht_ps[:], in_=h_t, identity=ident[:])
                nc.vector.tensor_copy(out=htall[:, t * 128:(t + 1) * 128],
                                      in_=ht_ps[:])

            relu = mybir.ActivationFunctionType.Relu
            for t in range(tiles):
                c_ps = ps.tile([128, F], f32, tag="cps", bufs=3)
                nc.tensor.matmul(c_ps[:], htall[:, t * 128:(t + 1) * 128],
                                 wat_sb[:], start=True, stop=False)
                nc.tensor.matmul(c_ps[:], ones1[:], ba_sb[:],
                                 start=False, stop=True)
                nc.scalar.activation(out=out_sb[:, t * F:(t + 1) * F],
                                     in_=c_ps[:], func=relu)

            # ======== phase C: combine counts, mask, store ========
            with tc.high_priority(offset=-(1 << 20)):
                out4 = out_sb[:].rearrange("p (u v f) -> p u v f", v=2, f=F)
                od4 = out_d[:].rearrange("p (u v f) -> p u v f", v=2, f=F)
                masks = []
                for par in range(2):
                    s0 = sbt.tile([128, GRP + 1], f32, tag=f"s0{par}")
                    s1 = sbt.tile([128, GRP + 1], f32, tag=f"s1{par}")
                    msk = sbt.tile([128, GRP], IODT, tag=f"m{par}")
                    # structural ordering guard: pre-write the accumulator
                    # sum tile from out_sb's tail (written by the LAST dense
                    # activations) so no scheduler model can hoist this
                    # chain ahead of the dense work in the in-order streams
                    nc.vector.tensor_copy(out=s0[:],
                                          in_=out_sb[:, -(GRP + 1):])
                    t0, t1, t2, t3 = [acc[r][par] for r in range(nrot)]
                    nc.vector.tensor_tensor(out=s0[:], in0=t0[:], in1=t1[:],
                                            op=mybir.AluOpType.add)
                    nc.vector.tensor_tensor(out=s1[:], in0=t2[:], in1=t3[:],
                                            op=mybir.AluOpType.add)
                    nc.vector.tensor_tensor(out=s0[:], in0=s0[:], in1=s1[:],
                                            op=mybir.AluOpType.add)
                    nc.vector.tensor_scalar_min(out=msk[:], in0=s0[:, :GRP],
                                                scalar1=1.0)
                    masks.append(msk)
                # per-parity strided multiplies in three group ranges so
                # each range's contiguous store pipelines with later
                # multiplies; the Pool engine (idle after scatters, ~2x
                # slower per column than DVE) takes two odd ranges so both
                # engines finish together
                for ri, (g0, g1) in enumerate(((0, 16), (16, 32),
                                               (32, GRP))):
                    for par in range(2):
                        eng = (nc.gpsimd if (par == 1 and ri > 0)
                               else nc.vector)
                        eng.tensor_tensor(
                            out=out4[:, g0:g1, par, :],
                            in0=out4[:, g0:g1, par, :],
                            in1=masks[par][:, g0:g1][:, :, None]
                                .to_broadcast([128, g1 - g0, F]),
                            op=mybir.AluOpType.mult)
                    nc.sync.dma_start(out=od4[:, g0:g1, :, :],
                                      in_=out4[:, g0:g1, :, :])

    nc.compile()
    return nc


_cache = {}


def _get_nc(cfg: Cfg = FULL):
    key = (cfg.cap, cfg.chunk)
    if key not in _cache:
        _cache[key] = build(cfg)
    return _cache[key]


def _in_maps(cfg: Cfg, triplets, ent_embed, W_a, b_a):
    src = np.ascontiguousarray(np.asarray(triplets)[:, 0]).astype(np.int64)
    bucket = src // NPC
    local = (src - bucket * NPC).astype(np.int16)
    counts = np.bincount(bucket, minlength=N_CORES)
    order = np.argsort(bucket, kind="stable")
    ls = local[order]
    offs = np.zeros(N_CORES + 1, np.int64)
    np.cumsum(counts, out=offs[1:])

    n = ent_embed.shape[0]
    emb_pad = np.zeros((cfg.np_total, F), np.float32)
    emb_pad[:n] = np.asarray(ent_embed, np.float32)
    bft = mybir.dt.np(IODT)
    wa = np.ascontiguousarray(np.asarray(W_a, np.float32).T).astype(bft)
    ba = np.asarray(b_a, np.float32).reshape(1, F).astype(bft)

    maps = []
    for c in range(N_CORES):
        s = ls[offs[c]:offs[c + 1]]
        assert s.shape[0] <= cfg.cap, "bucket overflow; rebuild larger"
        sp = np.empty(cfg.cap, np.int16)
        sp[:s.shape[0]] = s
        npad = cfg.cap - s.shape[0]
        if npad:
            # dump idxs: slot 98 -> group col GRP, spread over partitions
            sp[s.shape[0]:] = (NPC + ((c * 31 + np.arange(npad)) % 128)
                               ).astype(np.int16)
        blocks = []
        off = 0
        for sz in cfg.chunks:
            blocks.append(sp[off:off + sz].reshape(16, sz // 16))
            off += sz
        ids = np.concatenate(blocks, axis=1)
        # node r at partition r%128, tile col r//128
        emb_c = emb_pad[c * NPC:(c + 1) * NPC]
        emb_l = emb_c.reshape(cfg.tiles, 128, F).transpose(1, 0, 2)
        maps.append({
            "ids": np.ascontiguousarray(ids),
            "emb": np.ascontiguousarray(
                emb_l.reshape(128, cfg.tiles * F)).astype(bft),
            "wa": wa,
            "ba": ba,
        })
    return maps


def kernel(triplets, ent_embed, W_a, b_a, W_a2, b_a2):
    src64 = np.asarray(np.asarray(triplets)[:, 0], np.int64)
    counts = np.bincount(src64 // NPC, minlength=N_CORES)
    cfg = FULL
    if counts.max() > cfg.cap:
        cfg = Cfg(k=int(np.ceil(counts.max() / FULL.chunk)), tail=0)
    nc = _get_nc(cfg)
    maps = _in_maps(cfg, triplets, ent_embed, W_a, b_a)
    res = run_bass_kernel_spmd(nc, maps, core_ids=list(range(N_CORES)))
    outs = []
    for r in res.results:
        o = np.asarray(r["out"]).astype(np.float32).reshape(128, cfg.tiles,
                                                            F)
        outs.append(o.transpose(1, 0, 2).reshape(NPC, F))
    out = np.concatenate(outs, axis=0)
    return np.ascontiguousarray(out[:ent_embed.shape[0]])

